# revision 53
# baseline (speedup 1.0000x reference)
"""Trainium2 Bass kernel for nn_BoTorchGPWorldModel (GP moment-matching world model).

Math restructuring: for every output-pair (a,c) and particle b, the N x N kernel
matrix is Q = exp(X G X^T + row (x) 1 + 1 (x) col) with a host-precomputed
12x12 G = diag(g) - diag(h_a) S diag(h_c), S = (Sigma_b + diag(Lam))^-1, and
[N] row/col vectors (c_ab folded in via logs). The device computes, per
(pair, b): two chained PE matmuls (row/col folded in as extra contraction
rows) -> exp on ACT -> beta_a^T Q (PE, masked accumulation) and, for diagonal
pairs, sum(inv_K^T * Q) (DVE) reduced across partitions on GPSIMD.

Sharding: 36 unique pairs (symmetry) over 8 cores; core k owns diag pair (k,k)
in slot 0 plus 3-4 off-diagonal pairs (5 uniform slots, zero-padded).
All small O(D^3)/O(N D^2) linear algebra (12x12 inverses, dets, the
predictive-mean path) runs on host in float64.

Hardware constraint honored throughout: each engine instruction may carry at
most ONE sync-wait, so every instruction's cross-engine dependencies are
arranged to collapse onto a single foreign engine (DMA-fed operands are
re-produced on the right engine first).
"""
import numpy as np

E, A, NT, B = 8, 4, 128, 8
D = E + A            # 12
NSLOT = 5            # pair slots per core; slot 0 = diagonal pair
NIND = 4             # per-beta-block indicator rows (fold row-vecs into mm2)
K1 = D + NSLOT + NIND  # 21: mm1 contraction (X^T + per-slot col + indicators)
K2 = D + 1 + NIND      # 17: mm2 contraction (X^T + ones + row-vec rows)
# PE lhsT/rhs base partition must be 0/32/64 -> slot chunks live at those bases:
# tile A holds slots 0-2 (bases 0/32/64, 64+17=81 rows), tile B slots 3-4 (0/32)
M1A, M1B = 81, 49
M1 = M1A + M1B       # 130: mm1 output rows per particle (with gaps)
SLOT_LOC = [(0, 0), (0, 32), (0, 64), (1, 0), (1, 32)]  # slot -> (tile, base)
NCORES = 8

F32R = True          # TF32-like PE fast path (4x over f32 at N>=256)

_PROGRAM = None


def _core_pairs():
    off = [(a, c) for a in range(E) for c in range(a + 1, E)]
    cp = {k: [(k, k)] for k in range(NCORES)}
    for i, p in enumerate(off):
        cp[i % NCORES].append(p)
    return cp


def _build_program():
    global _PROGRAM
    if _PROGRAM is not None:
        return _PROGRAM
    import concourse.bass as bass
    import concourse.mybir as mybir
    from concourse import tile

    f32 = mybir.dt.float32
    fR = mybir.dt.float32r if F32R else f32
    nc = bass.Bass()

    W1 = B * M1 + B * NT          # 1040 lhs1 cols + 1024 rhs1 cols
    mm1in_d = nc.declare_dram_parameter("mm1in", [K1, W1], fR, isOutput=False)
    rowaug_d = nc.declare_dram_parameter("rowaug", [M1A, 512], fR, isOutput=False)
    bik_d = nc.declare_dram_parameter("bik", [NT, NSLOT * NSLOT + NT], f32, isOutput=False)
    outa_d = nc.declare_dram_parameter("out_all", [NSLOT, 1040], f32, isOutput=True)

    with tile.TileContext(nc) as tc:
        with (
            tc.tile_pool(name="const", bufs=1) as const,
            tc.tile_pool(name="t2ps", bufs=1, space="PSUM") as t2ps,
            tc.tile_pool(name="argps", bufs=3, space="PSUM") as argps,
            tc.tile_pool(name="yps", bufs=2, space="PSUM") as ypsp,
            tc.tile_pool(name="trps", bufs=1, space="PSUM") as trps,
            tc.tile_pool(name="qpool", bufs=3) as qpool,
            tc.tile_pool(name="qdiag", bufs=2) as qdiag,
            tc.tile_pool(name="qscr", bufs=8) as qscr,
            tc.tile_pool(name="fin", bufs=2) as fin,
        ):
            mm1in = const.tile([K1, W1], fR)
            rowaug = const.tile([M1A, 512], fR)
            bik = const.tile([NT, NSLOT * NSLOT + NT], f32)
            t2sbA = const.tile([M1A, B * NT], fR)
            t2sbB = const.tile([M1B, B * NT], fR)
            tcol = const.tile([NT, B], f32)

            lhs1 = mm1in[:, :B * M1]
            rhs1 = mm1in[:, B * M1:]

            for t, dsrc in ((mm1in, mm1in_d), (rowaug, rowaug_d),
                            (bik, bik_d)):
                nc.sync.dma_start(t[:], dsrc[:])

            # single-wait discipline: re-produce DMA-fed operands on the engine
            # whose semaphore the consumer already waits on.
            outsb = const.tile([NSLOT, 1040], f32)
            nc.vector.memset(outsb[:, 512:520], 0.0)
            rowaug2 = const.tile([M1A, 512], fR)   # DVE (mm2 waits DVE for t2sb)
            ba2 = const.tile([NT, NSLOT * NSLOT], fR)  # ACT (mm3 waits ACT for q)
            ik2 = const.tile([NT, NT], f32)        # ACT (ttr waits ACT for q)
            nc.vector.tensor_copy(rowaug2[:], rowaug[:])
            nc.scalar.copy(ba2[:], bik[:, :NSLOT * NSLOT])
            nc.scalar.copy(ik2[:], bik[:, NSLOT * NSLOT:])

            # mm1 per particle b: chunk rows = (G_sb @ X^T ; col_sb ; indicators)
            for b in range(B):
                t2pA = t2ps.tile([M1A, NT], f32, tag="t2pA")
                nc.tensor.matmul(
                    t2pA[:],
                    lhs1[:, b * M1:b * M1 + M1A],
                    rhs1[:, b * NT:(b + 1) * NT],
                    start=True, stop=True,
                )
                nc.vector.tensor_copy(t2sbA[:, b * NT:(b + 1) * NT], t2pA[:])
                t2pB = t2ps.tile([M1B, NT], f32, tag="t2pB")
                nc.tensor.matmul(
                    t2pB[:],
                    lhs1[:, b * M1 + M1A:(b + 1) * M1],
                    rhs1[:, b * NT:(b + 1) * NT],
                    start=True, stop=True,
                )
                nc.vector.tensor_copy(t2sbB[:, b * NT:(b + 1) * NT], t2pB[:])

            # PE anchor: consume the last t2sb copies once so every later mm2
            # inherits the DVE clock via PE program order (single-wait limit).
            anchor = trps.tile([B, B], f32, tag="trx")
            nc.tensor.matmul(
                anchor[:],
                t2sbA[0:1, NT - 1::NT],      # one element from each A copy block
                t2sbB[0:1, NT - 1::NT],      # one element from each B copy block
                start=True, stop=True,
            )

            for h in range(2):
                yp5 = ypsp.tile([NSLOT, 512], f32)
                for s in range(NSLOT):
                    ti, base = SLOT_LOC[s]
                    t2src = t2sbA if ti == 0 else t2sbB
                    tioff = 0 if ti == 0 else 256
                    argp = argps.tile([NT, 512], f32)
                    nc.tensor.matmul(
                        argp[:],
                        rowaug2[base:base + K2, tioff + h * NT:tioff + (h + 1) * NT],
                        t2src[base:base + K2, h * 512:(h + 1) * 512],
                        start=True, stop=True,
                    )
                    q = (qdiag if s == 0 else qpool).tile([NT, 512], fR)
                    for qq in range(4):
                        nc.scalar.activation(
                            q[:, qq * NT:(qq + 1) * NT],
                            argp[:, qq * NT:(qq + 1) * NT],
                            bass.mybir.ActivationFunctionType.Exp,
                        )
                    # accumulate y_s = beta_a(s)^T Q into row s of yp5 via a
                    # masked lhsT (beta in column s, zeros elsewhere)
                    nc.tensor.matmul(
                        yp5[:], ba2[:, s * NSLOT:(s + 1) * NSLOT], q[:],
                        start=(s == 0), stop=(s == NSLOT - 1),
                    )
                    if s == 0:
                        for qq in range(4):
                            bidx = h * 4 + qq
                            qsc = qscr.tile([NT, NT], f32)
                            nc.vector.tensor_mul(
                                qsc[:], q[:, qq * NT:(qq + 1) * NT], ik2[:])
                            nc.vector.tensor_reduce(
                                out=tcol[:, bidx:bidx + 1],
                                in_=qsc[:],
                                axis=bass.mybir.AxisListType.X,
                                op=bass.mybir.AluOpType.add,
                            )
                nc.vector.tensor_copy(
                    outsb[:, h * 520:h * 520 + 512], yp5[:])

            ones_const = nc.const_aps.aps[(f32, 1.0)]   # init-time memset, no dep
            trp = trps.tile([1, B], f32, tag="trx")
            nc.tensor.matmul(trp[:], ones_const, tcol[:], start=True, stop=True)
            nc.vector.tensor_copy(outsb[0:1, 512:512 + B], trp[:])
            nc.sync.dma_start(outa_d[:], outsb[:])

    # This walrus build accepts at most ONE sync-wait per instruction; split
    # extras onto same-engine NOPs placed immediately before the instruction.
    for f in nc.m.functions:
        for blk in f.blocks:
            new = []
            for inst in blk.instructions:
                si = inst.sync_info
                if si is not None and si.on_wait and len(si.on_wait) > 1:
                    waits = list(si.on_wait)
                    for w in waits[:-1]:
                        nop = mybir.InstNoOp(
                            name=nc.get_next_instruction_name(),
                            engine=inst.engine,
                            sync_info=mybir.SyncInfo(on_wait=[w], on_update=[]),
                            bass_nofuse=True,
                        )
                        nc.register_instruction(nop, overwrite=True)
                        new.append(nop)
                    inst.sync_info = mybir.SyncInfo(
                        on_wait=[waits[-1]], on_update=list(si.on_update or []))
                new.append(inst)
            try:
                blk.instructions = new
            except Exception:
                blk.instructions[:] = new

    _PROGRAM = nc
    return nc


def _host_precompute(inputs):
    f = np.float64
    m_x = np.asarray(inputs["m_x"], f)
    s_x = np.asarray(inputs["s_x"], f)
    m_u = np.asarray(inputs["m_u"], f)
    s_u = np.asarray(inputs["s_u"], f)
    c_xu = np.asarray(inputs["c_xu"], f)
    X = np.asarray(inputs["X_train"], f)
    ls = np.asarray(inputs["lengthscales"], f)
    var_v = np.asarray(inputs["variances"], f)[:, 0]
    inv_K = np.asarray(inputs["inv_K"], f)
    beta = np.asarray(inputs["beta"], f)

    mu = np.concatenate([m_x, m_u], -1)
    s_ = s_x @ c_xu
    upper = np.concatenate([s_x, s_], -1)
    lower = np.concatenate([np.swapaxes(s_, -1, -2), s_u], -1)
    Sig = np.concatenate([upper, lower], -2)
    l2 = ls ** 2
    inv_l2 = 1.0 / l2

    cp = _core_pairs()
    in_maps = []
    for k in range(NCORES):
        lhs1 = np.zeros((K1, B * M1), np.float32)
        rhs1 = np.zeros((K1, B * NT), np.float32)
        rowaug = np.zeros((M1A, 512), np.float32)
        ba = np.zeros((NT, NSLOT * NSLOT), np.float32)
        for b in range(B):
            rhs1[:D, b * NT:(b + 1) * NT] = X.T
            rhs1[D + NSLOT + (b % 4), b * NT:(b + 1) * NT] = 1.0
        for s, (a, c) in enumerate(cp[k]):
            inv_sum = inv_l2[a] + inv_l2[c]
            Lam = 1.0 / inv_sum
            g = 1.0 / (l2[a] + l2[c])
            h_a = Lam * inv_l2[a]
            h_c = Lam * inv_l2[c]
            p = (X ** 2) @ g
            U = X * h_a
            V = X * h_c
            ba[:, s * NSLOT + s] = beta[a]
            ti, base = SLOT_LOC[s]
            tioff = 0 if ti == 0 else 256
            for hh in range(2):
                blk = rowaug[base:base + K2, tioff + hh * NT:tioff + (hh + 1) * NT]
                blk[:D] = X.T
                blk[D] = 1.0
            for b in range(B):
                S = np.linalg.inv(Sig[b] + np.diag(Lam))
                R_ab = Sig[b] * inv_sum[None, :] + np.eye(D)
                ln_c = np.log(var_v[a] * var_v[c]) - 0.5 * np.log(np.linalg.det(R_ab))
                G = np.diag(g) - (h_a[:, None] * S) * h_c[None, :]
                Smu = S @ mu[b]
                msm = mu[b] @ Smu
                row = -0.5 * p - 0.5 * np.sum((U @ S) * U, -1) + U @ Smu - 0.25 * msm + 0.5 * ln_c
                col = -0.5 * p - 0.5 * np.sum((V @ S) * V, -1) + V @ Smu - 0.25 * msm + 0.5 * ln_c
                cb = b * M1 + (M1A if ti else 0) + base
                lhs1[0:D, cb:cb + D] = G.T
                lhs1[D + s, cb + D] = 1.0
                lhs1[D + NSLOT + (b % 4), cb + D + 1 + (b % 4)] = 1.0
                rhs1[D + s, b * NT:(b + 1) * NT] = col
                hh, q4 = b // 4, b % 4
                rowaug[base + D + 1 + q4,
                       tioff + hh * NT:tioff + (hh + 1) * NT] = row
        in_maps.append(dict(
            mm1in=np.concatenate([lhs1, rhs1], axis=1),
            rowaug=rowaug,
            bik=np.concatenate(
                [ba, inv_K[k].T.astype(np.float32)], axis=1),
        ))

    ctx = dict(inputs=inputs, mu=mu, Sig=Sig, upper=upper, X=X, ls=ls,
               var_v=var_v, beta=beta, cp=cp)
    return in_maps, ctx


def _assemble(ctx, results):
    f = np.float64
    inputs = ctx["inputs"]
    mu, Sig, upper, X = ctx["mu"], ctx["Sig"], ctx["upper"], ctx["X"]
    ls, var_v, beta, cp = ctx["ls"], ctx["var_v"], ctx["beta"], ctx["cp"]
    m_x = np.asarray(inputs["m_x"], f)
    s_x = np.asarray(inputs["s_x"], f)
    noises = np.asarray(inputs["noises"], f)

    main = np.zeros((E, E, B), f)
    trace_val = np.zeros((E, B), f)
    for k in range(NCORES):
        oa = np.asarray(results[k]["out_all"], f)      # [5,1040]
        trace_val[k] = oa[0, 512:512 + B]
        for s, (a, c) in enumerate(cp[k]):
            for b in range(B):
                off = (b // 4) * 520 + (b % 4) * NT
                y = oa[s, off:off + NT]
                v = y @ beta[c]
                main[a, c, b] = v
                main[c, a, b] = v

    # mean path (host, f64)
    inv_ls = 1.0 / ls
    inp = X[None] - mu[:, None]
    inv_N = inp[:, None] * inv_ls[None, :, None]
    B_mat = inv_ls[None, :, :, None] * Sig[:, None] * inv_ls[None, :, None, :] + np.eye(D)
    t = inv_N @ np.swapaxes(np.linalg.inv(B_mat), -1, -2)
    se = np.exp(-0.5 * np.sum(inv_N * t, -1))
    lb = se * beta[None]
    c = var_v[None] / np.sqrt(np.linalg.det(B_mat))
    pred_mean = np.sum(lb, -1) * c
    t_inv_L = t * inv_ls[None, :, None, :]
    cross_cov = np.swapaxes(np.einsum('bend,ben->bed', t_inv_L, lb) * c[..., None], -1, -2)

    pred_cov = np.moveaxis(main, -1, 0)
    diag_corr = var_v[:, None] - trace_val + noises[:, None]
    pred_cov = pred_cov + diag_corr.T[:, :, None] * np.eye(E)
    pred_cov = pred_cov - pred_mean[:, :, None] * pred_mean[:, None, :]
    pred_cov = 0.5 * (pred_cov + np.swapaxes(pred_cov, -1, -2))
    cov_xf = upper @ cross_cov
    m_out = m_x + pred_mean
    s_out = s_x + pred_cov + cov_xf + np.swapaxes(cov_xf, -1, -2)
    s_out = 0.5 * (s_out + np.swapaxes(s_out, -1, -2)) + 1e-8 * np.eye(E)
    return m_out.astype(np.float32), s_out.astype(np.float32)


def run_on_device(in_maps, trace=False):
    from concourse.bass_utils import run_bass_kernel_spmd
    nc = _build_program()
    res = run_bass_kernel_spmd(nc, in_maps, list(range(NCORES)), trace=trace)
    return res


def kernel(**inputs):
    in_maps, ctx = _host_precompute(inputs)
    res = run_on_device(in_maps)
    return _assemble(ctx, res.results)
